# revision 1
# baseline (speedup 1.0000x reference)
"""Differential attention kernel for 8 TRN2 NeuronCores.

Strategy: data-parallel over the 16384 tokens (2048/core).  Per core:
  - PE-transpose x and the four weight matrices into contraction-major
    (bf16) layout.
  - Four 2048x2048 projections run on the TensorEngine in bf16
    (fp32 PSUM accumulate).
  - The per-token 32x32 head attention (q/k gram, softmax, diff, @v,
    RMS norm) runs on the Vector/GpSimd/Scalar engines with tokens on
    partitions, overlapped with PE work via the Tile scheduler.
"""

import numpy as np

DIM = 2048
H = 16
D = 64
LAYER_NUM = 12
LAM_INIT = 0.8 - 0.6 * float(np.exp(-0.3 * LAYER_NUM))
SCALE = D ** -0.5
EPS = 1e-5

NCORES = 8
T = 2048                  # tokens per core
NT = T // 128             # 16 token tiles per core
P = 128
OC = 4                    # output chunks of 512 per projection
KT = DIM // P             # 16 contraction tiles

_PROGRAM_CACHE = {}


def _build(lam: float):
    import concourse.bass as bass
    import concourse.tile as tile
    from concourse import bacc
    from concourse import mybir
    from concourse.masks import make_identity

    f32 = mybir.dt.float32
    bf16 = mybir.dt.bfloat16
    Alu = mybir.AluOpType
    Act = mybir.ActivationFunctionType
    Axis = mybir.AxisListType

    nc = bacc.Bacc("TRN2", target_bir_lowering=False)

    x_d = nc.dram_tensor("x", [T, DIM], f32, kind="ExternalInput")
    wq_d = nc.dram_tensor("Wq", [DIM, DIM], f32, kind="ExternalInput")
    wk_d = nc.dram_tensor("Wk", [DIM, DIM], f32, kind="ExternalInput")
    wv_d = nc.dram_tensor("Wv", [DIM, DIM], f32, kind="ExternalInput")
    wo_d = nc.dram_tensor("Wo", [DIM, DIM], f32, kind="ExternalInput")
    bo_d = nc.dram_tensor("bo", [P, DIM], f32, kind="ExternalInput")
    y_d = nc.dram_tensor("y", [T, DIM], f32, kind="ExternalOutput")

    with tile.TileContext(nc) as tc:
        with (
            tc.tile_pool(name="persist", bufs=1) as persist,
            tc.tile_pool(name="wpool", bufs=1) as wpool,
            tc.tile_pool(name="temps", bufs=2) as temps,
            tc.tile_pool(name="cast", bufs=3) as cast_p,
            tc.tile_pool(name="psum_mm", bufs=3, space="PSUM") as psum_mm,
            tc.tile_pool(name="psum_tr", bufs=3, space="PSUM") as psum_tr,
            tc.tile_pool(name="dram", bufs=1, space="DRAM") as dram,
        ):
            ident = persist.tile([P, P], bf16)
            make_identity(nc, ident)

            eps_sb = persist.tile([P, 1], f32)
            nc.vector.memset(eps_sb, EPS)

            # bias pre-replicated host-side to [P, DIM]
            bo_sb = persist.tile([P, DIM], f32)
            nc.sync.dma_start(out=bo_sb, in_=bo_d[:, :])

            # ---- transpose helper: DRAM [r, c] f32 -> SBUF [128, c/128, r] bf16
            def load_transposed(src_d, dst_sb, scale=None):
                for r in range(KT):
                    wf = temps.tile([P, DIM], f32, tag="ldT_f32")
                    nc.sync.dma_start(out=wf, in_=src_d[r * P:(r + 1) * P, :])
                    wb = cast_p.tile([P, DIM], bf16, tag="ldT_bf")
                    if scale is None:
                        nc.vector.tensor_copy(out=wb, in_=wf)
                    else:
                        nc.vector.tensor_scalar_mul(wb, wf, scale)
                    for c in range(KT):
                        ps = psum_tr.tile([P, P], bf16, tag="trps")
                        nc.tensor.transpose(
                            ps, wb[:, c * P:(c + 1) * P], ident)
                        nc.vector.tensor_copy(
                            out=dst_sb[:, c, r * P:(r + 1) * P], in_=ps)

            # DRAM spill for q/k/v (bf16, token-major)
            q_spill = dram.tile([T, DIM], bf16)
            k_spill = dram.tile([T, DIM], bf16)
            v_spill = dram.tile([T, DIM], bf16)

            # ---- projection: y[t,o] = sum_i x[t,i] W[o,i]
            def project(xT, w_dram, sink, scale=None):
                wT = wpool.tile([P, KT, DIM], bf16, tag="wT")
                load_transposed(w_dram, wT, scale=scale)
                for tt in range(NT):
                    for oc in range(OC):
                        ps = psum_mm.tile([P, 512], f32, tag="mmps")
                        for kt in range(KT):
                            nc.tensor.matmul(
                                ps,
                                lhsT=xT[:, kt, tt * P:(tt + 1) * P],
                                rhs=wT[:, kt, oc * 512:(oc + 1) * 512],
                                start=(kt == 0), stop=(kt == KT - 1))
                        sink(tt, oc, ps)

            def spill_sink(dst):
                def _sink(tt, oc, ps):
                    sb = cast_p.tile([P, 512], bf16, tag="spill")
                    nc.vector.tensor_copy(out=sb, in_=ps)
                    nc.sync.dma_start(
                        out=dst[tt * P:(tt + 1) * P, oc * 512:(oc + 1) * 512],
                        in_=sb)
                return _sink

            with tc.tile_pool(name="xT_pool", bufs=1) as xT_pool:
                xT = xT_pool.tile([P, KT, T], bf16)      # x.T  (i on part)
                load_transposed(x_d, xT)
                # q has SCALE folded in host-side (Wq pre-scaled)
                project(xT, wq_d, spill_sink(q_spill))
                project(xT, wk_d, spill_sink(k_spill))
                project(xT, wv_d, spill_sink(v_spill))

            # Wo transposed, resident for the output projection
            woT = wpool.tile([P, KT, DIM], bf16, tag="wT")
            load_transposed(wo_d, woT)

            # ---- attention per 128-token tile ----
            inv2d = 1.0 / (2.0 * D)
            one_m_lam_init = 1.0 - LAM_INIT

            with (
                tc.tile_pool(name="attn2", bufs=2) as attn_p,
                tc.tile_pool(name="attn1", bufs=1) as attn_s,
            ):
                for tt in range(NT):
                    q_t = attn_p.tile([P, DIM], bf16, tag="q_t")
                    k_t = attn_p.tile([P, DIM], bf16, tag="k_t")
                    v_t = attn_p.tile([P, DIM], bf16, tag="v_t")
                    nc.sync.dma_start(
                        out=q_t, in_=q_spill[tt * P:(tt + 1) * P, :])
                    nc.sync.dma_start(
                        out=k_t, in_=k_spill[tt * P:(tt + 1) * P, :])
                    nc.sync.dma_start(
                        out=v_t, in_=v_spill[tt * P:(tt + 1) * P, :])

                    # deinterleaved views: [p, qi(2), h(16), d(64)]
                    q_v = q_t.rearrange("p (h d q) -> p q h d", d=D, q=2)
                    k_v = k_t.rearrange("p (h d q) -> p q h d", d=D, q=2)

                    # s[t, i, j] = sum_d q[t,i,:] k[t,j,:]  (i,j in [0,32))
                    s_all = attn_s.tile([P, 32, 32], f32, tag="s_all")
                    for j in range(32):
                        qi_j, h_j = divmod(j, H)
                        eng = nc.gpsimd if (j % 3 == 2) else nc.vector
                        prod = attn_p.tile([P, 2, H, D], bf16, tag="prod")
                        kb = k_v[:, qi_j:qi_j + 1, h_j:h_j + 1, :]
                        eng.tensor_tensor(
                            out=prod, in0=q_v,
                            in1=kb.to_broadcast([P, 2, H, D]), op=Alu.mult)
                        nc.vector.tensor_reduce(
                            out=s_all[:, :, j], in_=prod,
                            axis=Axis.X, op=Alu.add)

                    # s layout [p, i, j]; softmax over j; exp in place
                    nc.scalar.activation(
                        out=s_all.rearrange("p a b -> p (a b)"),
                        in_=s_all.rearrange("p a b -> p (a b)"), func=Act.Exp)

                    z = attn_s.tile([P, 32], f32, tag="z")
                    nc.vector.tensor_reduce(
                        out=z, in_=s_all, axis=Axis.X, op=Alu.add)
                    rz = attn_s.tile([P, 32], f32, tag="rz")
                    nc.vector.reciprocal(out=rz, in_=z)

                    # P[a,h] = e[a,h]*rz[a] - lam * e[16+a,16+h]*rz[16+a]
                    p1 = attn_s.tile([P, H, H], f32, tag="p1")
                    nc.vector.tensor_tensor(
                        out=p1, in0=s_all[:, 0:H, 0:H],
                        in1=rz[:, 0:H, None].to_broadcast([P, H, H]),
                        op=Alu.mult)
                    p2 = attn_s.tile([P, H, H], f32, tag="p2")
                    nc.vector.tensor_tensor(
                        out=p2, in0=s_all[:, H:32, H:32],
                        in1=rz[:, H:32, None].to_broadcast([P, H, H]),
                        op=Alu.mult)
                    pm = attn_s.tile([P, H, H], bf16, tag="pm")
                    nc.vector.scalar_tensor_tensor(
                        out=pm, in0=p2, scalar=-lam, in1=p1,
                        op0=Alu.mult, op1=Alu.add)

                    # u[t, a, e] = sum_h P[a,h] v[t, h, e]
                    v_r = v_t.rearrange("p (h e) -> p e h", h=H)  # [p,128,16]
                    u = attn_s.tile([P, H, 2 * D], f32, tag="u")
                    for ah in range(8):       # a in chunks of 2
                        wp = attn_p.tile([P, 2, 2 * D, H], bf16, tag="wprod")
                        eng = nc.gpsimd if (ah % 3 == 2) else nc.vector
                        eng.tensor_tensor(
                            out=wp,
                            in0=pm[:, 2 * ah:2 * ah + 2, None, :]
                                .to_broadcast([P, 2, 2 * D, H]),
                            in1=v_r[:, None, :, :]
                                .to_broadcast([P, 2, 2 * D, H]),
                            op=Alu.mult)
                        nc.vector.tensor_reduce(
                            out=u[:, 2 * ah:2 * ah + 2, :], in_=wp,
                            axis=Axis.X, op=Alu.add)

                    # RMS norm over e (2D=128) then * (1-LAM_INIT)
                    usq = attn_s.tile([P, H, 2 * D], bf16, tag="usq")
                    nc.scalar.activation(
                        out=usq.rearrange("p a e -> p (a e)"),
                        in_=u.rearrange("p a e -> p (a e)"), func=Act.Square)
                    m2 = attn_s.tile([P, H], f32, tag="m2")
                    nc.vector.tensor_reduce(
                        out=m2, in_=usq, axis=Axis.X, op=Alu.add)
                    sd = attn_s.tile([P, H], f32, tag="sd")
                    nc.scalar.activation(
                        out=sd, in_=m2, func=Act.Sqrt,
                        bias=eps_sb, scale=inv2d)
                    rstd = attn_s.tile([P, H], f32, tag="rstd")
                    nc.vector.reciprocal(out=rstd, in_=sd)

                    on_t = attn_s.tile([P, H, 2 * D], bf16, tag="on_t")
                    nc.vector.scalar_tensor_tensor(
                        out=on_t, in0=u, scalar=one_m_lam_init,
                        in1=rstd[:, :, None].to_broadcast([P, H, 2 * D]),
                        op0=Alu.mult, op1=Alu.mult)

                    # transpose out_n tile -> [feat, t] for Wo projection
                    onT = attn_s.tile([P, KT, P], bf16, tag="onT")
                    on_flat = on_t.rearrange("p a e -> p (a e)")
                    for c in range(KT):
                        ps = psum_tr.tile([P, P], bf16, tag="trps")
                        nc.tensor.transpose(
                            ps, on_flat[:, c * P:(c + 1) * P], ident)
                        nc.vector.tensor_copy(out=onT[:, c, :], in_=ps)

                    # y[tt] = out_n @ Wo.T + bo
                    y_sb = attn_p.tile([P, DIM], f32, tag="y_sb")
                    for oc in range(OC):
                        ps = psum_mm.tile([P, 512], f32, tag="mmps")
                        for kt in range(KT):
                            nc.tensor.matmul(
                                ps, lhsT=onT[:, kt, :],
                                rhs=woT[:, kt, oc * 512:(oc + 1) * 512],
                                start=(kt == 0), stop=(kt == KT - 1))
                        nc.vector.tensor_add(
                            out=y_sb[:, oc * 512:(oc + 1) * 512], in0=ps,
                            in1=bo_sb[:, oc * 512:(oc + 1) * 512])
                    nc.sync.dma_start(
                        out=y_d[tt * P:(tt + 1) * P, :], in_=y_sb)

    nc.finalize()
    return nc


def kernel(**inputs):
    x = np.asarray(inputs["x"], dtype=np.float32)
    Wq = np.asarray(inputs["Wq"], dtype=np.float32)
    Wk = np.asarray(inputs["Wk"], dtype=np.float32)
    Wv = np.asarray(inputs["Wv"], dtype=np.float32)
    Wo = np.asarray(inputs["Wo"], dtype=np.float32)
    bo = np.asarray(inputs["bo"], dtype=np.float32)
    lq1 = np.asarray(inputs["lq1"], dtype=np.float32)
    lq2 = np.asarray(inputs["lq2"], dtype=np.float32)
    lk1 = np.asarray(inputs["lk1"], dtype=np.float32)
    lk2 = np.asarray(inputs["lk2"], dtype=np.float32)

    lam = float(np.exp(np.sum(lq1 * lk1)) - np.exp(np.sum(lq2 * lk2))
                + LAM_INIT)

    b, n, _ = x.shape
    xt = np.ascontiguousarray(x.reshape(b * n, DIM))
    wq_s = np.ascontiguousarray(Wq * np.float32(SCALE))
    bo_rep = np.ascontiguousarray(np.broadcast_to(bo, (P, DIM)))

    key = round(lam, 6)
    if key not in _PROGRAM_CACHE:
        _PROGRAM_CACHE[key] = _build(lam)
    nc = _PROGRAM_CACHE[key]

    from concourse.bass_utils import run_bass_kernel_spmd

    in_maps = []
    for c in range(NCORES):
        in_maps.append({
            "x": np.ascontiguousarray(xt[c * T:(c + 1) * T]),
            "Wq": wq_s,
            "Wk": Wk,
            "Wv": Wv,
            "Wo": Wo,
            "bo": bo_rep,
        })

    res = run_bass_kernel_spmd(nc, in_maps, core_ids=list(range(NCORES)))
    y = np.concatenate([res.results[c]["y"] for c in range(NCORES)], axis=0)
    return y.reshape(b, n, DIM).astype(np.float32)

